# revision 1
# baseline (speedup 1.0000x reference)
"""Trainium2 Bass kernel for BasicQuantConv2d (sync-BN + HWGQ + gauss-quant + 3x3 conv).

Strategy (8 NeuronCores, data-parallel over batch):
  - Each core takes 4 of the 32 images: x shard [4, 128, 56, 56].
  - BN batch stats: per-core bn_stats/bn_aggr -> (mean, E[x^2])/8 payload,
    AllReduce across the 8 cores (sync-BN), then per-channel scale/bias.
  - BN + HWGQ folds to ia = RNE_round(clip(x*s_c + b_c, 0, 3)) in {0..3};
    RNE rounding via the fp32 magic constant 1.5*2^23 (matches jnp.round).
  - gauss_quantize(w) == iw * (step/2) with iw in {-3,-1,1,3}; std(w) is
    computed on-device (reduction + ones-matmul broadcast + Newton-refined
    rsqrt), weights transposed per-tap on the PE for the conv lhsT.
  - The 3x3 conv runs in fp8e4m3 (ia in {0..3}, iw in {-3,-1,1,3} are exact
    in fp8; PSUM accumulates fp32 => conv is EXACT integer arithmetic).
    Per output row-chunk: 3 DoubleRow matmuls (vertical tap pairs kh=0&1,
    pair-step 64B via the padded row width) + 3 plain fp8 matmuls (kh=2),
    accumulated across 6 groups into 7 PSUM banks per image.
  - ~100 tiny PE warm-up matmuls gated on the AllReduce result keep HAM at
    K=8/8 through the quantize window so the conv burst runs at 2.4 GHz.
  - Output = PSUM * (0.538*step/2) via ScalarE, DMA back per image.

`_build(n_iters=K)` emits the whole body K times straight-line (single
bass_exec NEFF) so test.py can measure per-iteration device time through the
~80ms axon RPC floor.
"""

import numpy as np

import concourse.bacc as bacc
import concourse.bass as bass
import concourse.tile as tile
from concourse import mybir
from concourse.masks import make_identity

N_CORES = 8
IMG = 4            # images per core
C = 128            # channels (= partitions)
HW = 56
S = HW * HW        # 3136 pixels per image
F = IMG * S        # 12544 columns per core
PR = 58            # padded rows
PCW = 64           # padded row width (interior at cols 2..57; pair-step 64B for DoubleRow)
R = 8              # output rows per matmul tile
NT = HW // R       # 7 row-chunks per image
NFREE = R * HW     # 448 matmul free dim

HWGQ_STEP = 0.538
GAUSS = 0.996
BN_EPS = 1e-3
MAGIC = float(np.float32(1.5 * 2**23))
NW = 128 * 128 * 9          # weight element count

_CACHE = {}


def _emit_body(nc, tc, pools, params, ablate=()):
    fp32 = mybir.dt.float32
    bf16 = mybir.dt.bfloat16
    fp8 = mybir.dt.float8e4
    xp, apadp, wp, tmpp, outp, smallp, psump, psmallp, dramp = pools
    x_d, gamma_d, beta_d, w_d, y_d = params
    AF = mybir.ActivationFunctionType
    OP = mybir.AluOpType

    # ---------------- load x (half-image granularity), stats ----------------
    SA = 4 * 448   # first 32 rows
    SB = 3 * 448   # last 24 rows
    xA = [xp.tile([C, SA], fp32, tag=f"xa{i}", name=f"xA{i}") for i in range(IMG)]
    xB = [xp.tile([C, SB], fp32, tag=f"xb{i}", name=f"xB{i}") for i in range(IMG)]
    for i in range(IMG):
        if "dma2" in ablate:
            nc.sync.dma_start(out=xA[i][:], in_=x_d.ap()[i][:, 0:SA])
            nc.scalar.dma_start(out=xB[i][:], in_=x_d.ap()[i][:, SA:S])
        else:
            nc.sync.dma_start(out=xA[i][:], in_=x_d.ap()[i][:, 0:SA])
            nc.sync.dma_start(out=xB[i][:], in_=x_d.ap()[i][:, SA:S])

    stats = smallp.tile([C, IMG * 7, 6], fp32)
    for i in range(IMG):
        ga = xA[i][:].rearrange("p (g f) -> p g f", g=4)
        gb = xB[i][:].rearrange("p (g f) -> p g f", g=3)
        for g in range(4):
            nc.vector.bn_stats(out=stats[:, i * 7 + g, :], in_=ga[:, g, :])
        for g in range(3):
            nc.vector.bn_stats(out=stats[:, i * 7 + 4 + g, :], in_=gb[:, g, :])
    mv = smallp.tile([C, 2], fp32)
    nc.vector.bn_aggr(out=mv[:], in_=stats[:])

    # payload: (mean/8, E[x^2]/8) ; E[x^2] = var + mean^2 in one fused op
    pay8 = smallp.tile([C, 2], fp32)
    ex2 = smallp.tile([C, 1], fp32)
    m2 = smallp.tile([C, 1], fp32)
    nc.vector.tensor_mul(m2[:], mv[:, 0:1], mv[:, 0:1])
    nc.vector.tensor_add(ex2[:], mv[:, 1:2], m2[:])
    nc.vector.tensor_scalar_mul(pay8[:, 0:1], mv[:, 0:1], 1.0 / N_CORES)
    nc.vector.tensor_scalar_mul(pay8[:, 1:2], ex2[:], 1.0 / N_CORES)

    # ---------------- weight path (overlaps loads/stats) ----------------
    w_sb = wp.tile([C, 128 * 9], fp32)
    nc.sync.dma_start(out=w_sb[:], in_=w_d.ap())

    ident = smallp.tile([C, 128], fp32)
    make_identity(nc, ident[:])

    # transpose each tap: wT[ci, slot, co]; slots pair (kh=0,kw) with (kh=1,kw)
    # adjacently for DoubleRow, kh=2 taps in slots 6..8.
    # slot order: (0,0),(1,0),(0,1),(1,1),(0,2),(1,2),(2,0),(2,1),(2,2)
    SLOT = {(0, 0): 0, (1, 0): 1, (0, 1): 2, (1, 1): 3,
            (0, 2): 4, (1, 2): 5, (2, 0): 6, (2, 1): 7, (2, 2): 8}
    wT = wp.tile([C, 9, 128], fp32)
    w3 = w_sb[:].rearrange("p (ci t) -> p ci t", t=9)
    for t in range(9):
        kh, kw = divmod(t, 3)
        pt = psmallp.tile([C, 128], fp32, tag="psm", name="pt")
        nc.tensor.transpose(pt[:], w3[:, :, t], ident[:])
        nc.scalar.copy(out=wT[:, SLOT[(kh, kw)], :], in_=pt[:])

    # global sum / sumsq of w: ScalarE accum_out row-sums + ones-matmul bcast
    w2_sb = wp.tile([C, 128 * 9], fp32)
    rsums = smallp.tile([C, 2], fp32)
    nc.scalar.activation(out=w2_sb[:], in_=w_sb[:], func=AF.Identity,
                         accum_out=rsums[:, 0:1])
    nc.scalar.activation(out=w2_sb[:], in_=w_sb[:], func=AF.Square,
                         accum_out=rsums[:, 1:2])
    ones = smallp.tile([C, 128], fp32)
    nc.vector.memset(ones[:], 1.0)
    pg = psmallp.tile([C, 128], fp32, tag="psm", name="pg")
    nc.tensor.matmul(pg[:, 0:2], lhsT=ones[:], rhs=rsums[:], start=True, stop=True)
    gs = smallp.tile([C, 2], fp32)
    nc.vector.tensor_copy(gs[:], pg[:, 0:2])

    # wvar = E[w^2] - E[w]^2 ; rw = rsqrt(wvar) Newton-refined
    wmean = smallp.tile([C, 1], fp32)
    wvar = smallp.tile([C, 1], fp32)
    nc.vector.tensor_scalar_mul(wmean[:], gs[:, 0:1], 1.0 / NW)
    nc.vector.tensor_scalar_mul(wvar[:], gs[:, 1:2], 1.0 / NW)
    wm2 = smallp.tile([C, 1], fp32)
    nc.vector.tensor_mul(wm2[:], wmean[:], wmean[:])
    nc.vector.tensor_sub(wvar[:], wvar[:], wm2[:])

    rw = smallp.tile([C, 1], fp32)
    nc.scalar.activation(out=rw[:], in_=wvar[:], func=AF.Sqrt)
    nc.vector.reciprocal(out=rw[:], in_=rw[:])
    tN = smallp.tile([C, 1], fp32)
    for _ in range(2):
        nc.vector.tensor_mul(tN[:], rw[:], rw[:])
        nc.vector.tensor_mul(tN[:], wvar[:], tN[:])
        nc.vector.tensor_scalar(tN[:], tN[:], -0.5, 1.5, OP.mult, OP.add)
        nc.vector.tensor_mul(rw[:], rw[:], tN[:])

    inv_step = smallp.tile([C, 1], fp32)
    nc.vector.tensor_scalar_mul(inv_step[:], rw[:], 1.0 / GAUSS)
    # alpha = 0.538 * step/2 = (0.538*0.996/2) * wvar * rw
    alpha = smallp.tile([C, 1], fp32)
    nc.vector.tensor_mul(alpha[:], wvar[:], rw[:])
    nc.vector.tensor_scalar_mul(alpha[:], alpha[:], HWGQ_STEP * GAUSS / 2.0)

    # quantize transposed weights -> iw in {-3,-1,1,3} (bf16)
    uw = wp.tile([C, 9, 128], fp32)
    nc.gpsimd.tensor_scalar(uw[:], wT[:], inv_step[:], 0.5, OP.mult, OP.add)
    nc.gpsimd.tensor_scalar(uw[:], uw[:], MAGIC, MAGIC, OP.add, OP.subtract)
    nc.gpsimd.tensor_scalar(uw[:], uw[:], 2.0, -1.0, OP.mult, OP.add)
    wq = wp.tile([C, 9, 128], fp8)
    nc.gpsimd.tensor_scalar(wq[:], uw[:], 3.0, -3.0, OP.min, OP.max)

    # ---------------- sync-BN all-reduce ----------------
    cc_in = dramp.tile([C, 2], fp32)
    cc_out = dramp.tile([C, 2], fp32)
    nc.sync.dma_start(out=cc_in[:], in_=pay8[:])
    if "noar" in ablate:
        nc.sync.dma_start(out=cc_out[:], in_=cc_in[:])
    else:
        nc.gpsimd.collective_compute(
            "AllReduce",
            OP.add,
            replica_groups=[list(range(N_CORES))],
            ins=[cc_in.opt()],
            outs=[cc_out.opt()],
        )
    g_sb = smallp.tile([C, 2], fp32)
    nc.sync.dma_start(out=g_sb[:], in_=cc_out[:])

    # PE warm-up during the post-AllReduce quantize window: ~100 tiny
    # matmuls dependent on g_sb keep/get HAM to K=8/8 before the conv burst.
    if "nowarm" not in ablate:
        ps_warm = psmallp.tile([C, 128], fp32, tag="psm", name="ps_warm")
        for _ in range(100):
            nc.tensor.matmul(ps_warm[0:32, 0:2], lhsT=ones[:, 0:32], rhs=g_sb[:],
                             start=True, stop=True)

    # ---------------- global scale/bias ----------------
    gb = smallp.tile([C, 2], fp32)
    gamma_ap = gamma_d.ap().rearrange("(p one) -> p one", one=1)
    beta_ap = beta_d.ap().rearrange("(p one) -> p one", one=1)
    nc.sync.dma_start(out=gb[:, 0:1], in_=gamma_ap)
    nc.sync.dma_start(out=gb[:, 1:2], in_=beta_ap)

    vge = smallp.tile([C, 1], fp32)   # var + eps
    gm2 = smallp.tile([C, 1], fp32)
    nc.vector.tensor_mul(gm2[:], g_sb[:, 0:1], g_sb[:, 0:1])
    nc.vector.tensor_sub(vge[:], g_sb[:, 1:2], gm2[:])
    nc.vector.tensor_scalar_add(vge[:], vge[:], BN_EPS)
    rx = smallp.tile([C, 1], fp32)
    nc.scalar.activation(out=rx[:], in_=vge[:], func=AF.Sqrt)
    nc.vector.reciprocal(out=rx[:], in_=rx[:])
    tX = smallp.tile([C, 1], fp32)
    for _ in range(2):
        nc.vector.tensor_mul(tX[:], rx[:], rx[:])
        nc.vector.tensor_mul(tX[:], vge[:], tX[:])
        nc.vector.tensor_scalar(tX[:], tX[:], -0.5, 1.5, OP.mult, OP.add)
        nc.vector.tensor_mul(rx[:], rx[:], tX[:])

    # s = gamma * rsqrt / 0.538 ; b = (beta - mean*gamma*rsqrt) / 0.538
    s_q = smallp.tile([C, 1], fp32)
    b_q = smallp.tile([C, 1], fp32)
    ta = smallp.tile([C, 1], fp32)
    nc.vector.tensor_mul(ta[:], gb[:, 0:1], rx[:])          # A = gamma*inv
    nc.vector.tensor_scalar_mul(s_q[:], ta[:], 1.0 / HWGQ_STEP)
    tb = smallp.tile([C, 1], fp32)
    nc.vector.tensor_mul(tb[:], g_sb[:, 0:1], ta[:])        # mean*A
    nc.vector.tensor_sub(tb[:], gb[:, 1:2], tb[:])          # beta - mean*A
    nc.vector.tensor_scalar_mul(b_q[:], tb[:], 1.0 / HWGQ_STEP)

    # ---------------- per-image quantize + conv ----------------
    a_t = [apadp.tile([C, PR, PCW], fp8, tag=f"a{i}", name=f"a_t{i}")
           for i in range(IMG)]
    for i in range(IMG):
        if "borders" in ablate:
            nc.gpsimd.memset(a_t[i][:, 0, :], 0.0)          # top pad row
            nc.gpsimd.memset(a_t[i][:, 57, :], 0.0)         # bottom pad row
            nc.gpsimd.memset(a_t[i][:, 1:57, 0:2], 0.0)     # left pad cols
            nc.gpsimd.memset(a_t[i][:, 1:57, 58:64], 0.0)   # right pad cols
        else:
            nc.gpsimd.memset(a_t[i][:], 0.0)

    for i in range(IMG):
        u_sb = tmpp.tile([C, S], fp32, tag="u", name=f"u_sb{i}")
        nc.scalar.activation(out=u_sb[:, 0:SA], in_=xA[i][:], func=AF.Identity,
                             bias=b_q[:], scale=s_q[:])
        nc.scalar.activation(out=u_sb[:, SA:S], in_=xB[i][:], func=AF.Identity,
                             bias=b_q[:], scale=s_q[:])
        c_sb = tmpp.tile([C, S], fp32, tag="c", name=f"c_sb{i}")
        for (r0, r1) in ((0, 16), (16, 32), (32, 48), (48, 56)):
            lo, hi = r0 * HW, r1 * HW
            nc.vector.tensor_scalar(c_sb[:, lo:hi], u_sb[:, lo:hi], 3.0, 0.0,
                                    OP.min, OP.max)
            nc.vector.tensor_scalar(a_t[i][:, r0 + 1:r1 + 1, 2:58],
                                    c_sb[:, lo:hi].rearrange(
                                        "p (h w) -> p h w", h=r1 - r0),
                                    MAGIC, MAGIC, OP.add, OP.subtract)

        out_sb = outp.tile([C, S], fp32, tag="o", name=f"out_sb{i}")
        base = a_t[i][:]
        ps = [psump.tile([C, NFREE], fp32, tag=f"ps{c}", name=f"ps{i}_{c}")
              for c in range(NT)]
        # groups: 3 DoubleRow pairs (kh=0&1 per kw), then 3 singles (kh=2)
        for g in range(6):
            for cix in range(NT):
                h0 = cix * R
                if g < 3:
                    kw = g
                    rhs = bass.AP(
                        tensor=base.tensor,
                        offset=base.offset + (h0 + 0) * PCW + (kw + 1),
                        ap=[base.ap[0], [PCW, 2], [PCW, R], [1, HW]],
                    )
                    if "noconv" in ablate:
                        continue
                    nc.tensor.matmul(ps[cix][:], lhsT=wq[:, 2 * kw: 2 * kw + 2, :],
                                     rhs=rhs, start=(g == 0), stop=(g == 5),
                                     perf_mode=mybir.MatmulPerfMode.DoubleRow)
                else:
                    kw = g - 3
                    if "noconv" in ablate:
                        continue
                    rhs = a_t[i][:, h0 + 2: h0 + 2 + R, kw + 1: kw + 1 + HW]
                    nc.tensor.matmul(ps[cix][:], lhsT=wq[:, 6 + kw, :], rhs=rhs,
                                     start=(g == 0), stop=(g == 5))
        for cix in range(NT):
            if "noconv" in ablate:
                continue
            h0 = cix * R
            nc.scalar.activation(out=out_sb[:, h0 * HW: (h0 + R) * HW],
                                 in_=ps[cix][:], func=AF.Identity, scale=alpha[:])
        if "noconv" in ablate:
            continue
        if i < IMG - 1:
            nc.sync.dma_start(out=y_d.ap()[i], in_=out_sb[:])
        else:
            for cix in range(NT):
                h0 = cix * R
                nc.sync.dma_start(out=y_d.ap()[i][:, h0 * HW:(h0 + R) * HW],
                                  in_=out_sb[:, h0 * HW:(h0 + R) * HW])


def _build(n_iters=1, ablate=()):
    fp32 = mybir.dt.float32

    nc = bacc.Bacc("TRN2", target_bir_lowering=False, debug=False,
                   num_devices=N_CORES)

    x_d = nc.declare_dram_parameter("x", [IMG, C, S], fp32, isOutput=False)
    gamma_d = nc.declare_dram_parameter("gamma", [C], fp32, isOutput=False)
    beta_d = nc.declare_dram_parameter("beta", [C], fp32, isOutput=False)
    w_d = nc.declare_dram_parameter("weight", [C, 128 * 9], fp32, isOutput=False)
    y_d = nc.declare_dram_parameter("y", [IMG, C, S], fp32, isOutput=True)
    params = (x_d, gamma_d, beta_d, w_d, y_d)

    with tile.TileContext(nc) as tc:
        with (
            tc.tile_pool(name="xp", bufs=1) as xp,
            tc.tile_pool(name="apad", bufs=1) as apadp,
            tc.tile_pool(name="wp", bufs=1) as wp,
            tc.tile_pool(name="tmp", bufs=2) as tmpp,
            tc.tile_pool(name="outp", bufs=2) as outp,
            tc.tile_pool(name="small", bufs=1) as smallp,
            tc.tile_pool(name="psum", bufs=1, space="PSUM") as psump,
            tc.tile_pool(name="psmall", bufs=1, space="PSUM") as psmallp,
            tc.tile_pool(name="dram", bufs=2, space="DRAM") as dramp,
        ):
            pools = (xp, apadp, wp, tmpp, outp, smallp, psump, psmallp, dramp)
            for _ in range(n_iters):
                _emit_body(nc, tc, pools, params, ablate)

    nc.finalize()
    return nc


def _get_nc(n_iters=1):
    key = ("nc", n_iters)
    if key not in _CACHE:
        _CACHE[key] = _build(n_iters)
    return _CACHE[key]


def make_in_maps(x, gamma, beta, weight):
    x = np.ascontiguousarray(np.asarray(x, np.float32)).reshape(N_CORES, IMG, C, S)
    w = np.ascontiguousarray(np.asarray(weight, np.float32)).reshape(C, 128 * 9)
    gamma = np.ascontiguousarray(np.asarray(gamma, np.float32))
    beta = np.ascontiguousarray(np.asarray(beta, np.float32))
    return [
        {"x": x[c], "gamma": gamma, "beta": beta, "weight": w}
        for c in range(N_CORES)
    ]


def kernel(x, gamma, beta, weight):
    import os
    from concourse.bass_utils import run_bass_kernel_spmd

    nc = _get_nc()
    in_maps = make_in_maps(x, gamma, beta, weight)
    core_ids = list(range(N_CORES))
    try:
        res = run_bass_kernel_spmd(nc, in_maps, core_ids)
    except ModuleNotFoundError:
        # BASS_TRACE set but no NTFF profile hook in this container
        os.environ["BASS_NEVER_TRACE"] = "1"
        res = run_bass_kernel_spmd(nc, in_maps, core_ids)
    out = np.stack([res.results[c]["y"] for c in range(N_CORES)], axis=0)
    return out.reshape(32, C, HW, HW).astype(np.float32)

